# revision 4
# baseline (speedup 1.0000x reference)
"""Trainium2 Bass kernel for BinarizeConv2d block:
   y = round(2*clip(BN(conv3x3(x, sign(w))), -1, 1))/2
Data-parallel over batch: 2 images per core on 8 NeuronCores.
"""
import sys
sys.path.insert(0, "/opt/trn_rl_repo")
import numpy as np
import ml_dtypes
import concourse.bass as bass
import concourse.bacc as bacc
import concourse.tile as tile
from concourse import mybir
from concourse.bass_utils import run_bass_kernel_spmd

F32 = mybir.dt.float32
BF16 = mybir.dt.bfloat16

N_CORES = 8
NPC = 2           # images per core
C = 32
H = W = 224
WP = 226          # padded width
NSUP = 7          # 8-output-row superblocks per quadrant (56 rows / 8)
MAGIC = 12582912.0  # 1.5 * 2**23 -> fp32 round-to-nearest-even trick
EPS = 1e-5
NTOT = float(N_CORES * NPC * H * W)  # elements per channel globally

_cache = {}


def _build_nc():
    nc = bacc.Bacc("TRN2", target_bir_lowering=False, debug=False,
                   num_devices=N_CORES)
    xh_ext = nc.declare_dram_parameter("xh", [NPC, C, H, W], BF16, isOutput=False)
    xl_ext = nc.declare_dram_parameter("xl", [NPC, C, H, W], BF16, isOutput=False)
    s_ext = nc.declare_dram_parameter("s", [128, 9, 128], BF16, isOutput=False)
    sel1_ext = nc.declare_dram_parameter("sel1", [128, 32], F32, isOutput=False)
    sel2_ext = nc.declare_dram_parameter("sel2", [32, 128], F32, isOutput=False)
    g_ext = nc.declare_dram_parameter("g", [32, 1], F32, isOutput=False)
    b_ext = nc.declare_dram_parameter("b", [32, 1], F32, isOutput=False)
    y_ext = nc.declare_dram_parameter("y", [NPC, C, H, W], BF16, isOutput=True)

    with tile.TileContext(nc) as tc:
        with (
            tc.tile_pool(name="big", bufs=1) as big,
            tc.tile_pool(name="small", bufs=1) as small,
            tc.tile_pool(name="ph2", bufs=2) as ph2,
            tc.tile_pool(name="psum", bufs=1, space="PSUM") as psum,
            tc.tile_pool(name="dram", bufs=1, space="DRAM") as dram,
        ):
            # ---- persistent SBUF tiles ----
            # x chunks: partition p = 32*q + ci ; free = (n, rowslot 10, WP)
            xhb = [big.tile([128, NPC, 10, WP], BF16, name=f"xhb{i}", tag=f"xh{i}") for i in range(2)]
            xlb = [big.tile([128, NPC, 10, WP], BF16, name=f"xlb{i}", tag=f"xl{i}") for i in range(2)]
            # y raw conv results: partition p = 32*j + co ; free = (n, q, s, i, w)
            y_raw = big.tile([128, NPC, 4, NSUP, 2, W], F32)
            s_sb = small.tile([128, 9, 128], BF16)
            stats_buf = small.tile([128, NSUP, 8, 6], F32)
            sel1_sb = small.tile([128, 32], F32)
            sel2_sb = small.tile([32, 128], F32)
            g_sb = small.tile([32, 1], F32)
            b_sb = small.tile([32, 1], F32)
            stats_sq = small.tile([128, 2], F32)
            stats_g = small.tile([128, 2], F32)
            msq_scr = small.tile([128, 112], F32)
            red = small.tile([128, 4], F32)  # cols: m_sum, msq_sum, ctv_sum
            t32 = small.tile([32, 2], F32)
            fin = small.tile([32, 8], F32)  # mean, msqm, var/v, rec, a, b, c, sc
            sb32 = small.tile([32, 2], F32)
            ab128 = small.tile([128, 2], F32)

            psum_t = psum.tile([128, 8, 512], F32)

            # ---- init ----
            for xb in (*xhb, *xlb):
                nc.vector.memset(xb[:], 0.0)
            nc.sync.dma_start(out=s_sb[:], in_=s_ext[:])
            nc.sync.dma_start(out=sel1_sb[:], in_=sel1_ext[:])
            nc.sync.dma_start(out=sel2_sb[:], in_=sel2_ext[:])
            nc.sync.dma_start(out=g_sb[:], in_=g_ext[:])
            nc.sync.dma_start(out=b_sb[:], in_=b_ext[:])

            pfull = psum_t[:]
            pstride = pfull.ap[0][0]

            # ---- phase 1: conv + stats per superblock ----
            for s in range(NSUP):
                xh_c, xl_c = xhb[s % 2], xlb[s % 2]
                if s == NSUP - 1:
                    # q=3 top halo row (global row 224) is stale from s-2: zero it
                    nc.vector.memset(xh_c[96:128, :, 9, :], 0.0)
                    nc.vector.memset(xl_c[96:128, :, 9, :], 0.0)
                for n in range(NPC):
                    for q in range(4):
                        r0 = 56 * q + 8 * s - 1
                        sl0 = 0
                        if r0 < 0:
                            sl0, r0 = 1, 0
                        r1 = min(r0 + 10 - sl0, H)
                        nr = r1 - r0
                        nc.sync.dma_start(
                            out=xh_c[32 * q:32 * q + 32, n, sl0:sl0 + nr, 1:225],
                            in_=xh_ext[n, :, r0:r1, :])
                        nc.sync.dma_start(
                            out=xl_c[32 * q:32 * q + 32, n, sl0:sl0 + nr, 1:225],
                            in_=xl_ext[n, :, r0:r1, :])
                xv_h = xh_c.rearrange("p n r w -> p n (r w)")
                xv_l = xl_c.rearrange("p n r w -> p n (r w)")
                for t in range(9):
                    kh, kw = divmod(t, 3)
                    for xv, hl in ((xv_h, 0), (xv_l, 1)):
                        for q in range(4):
                            for j in range(4):
                                off = (2 * j + kh) * WP + kw
                                for n in range(NPC):
                                    nc.tensor.matmul(
                                        psum_t[32 * j:32 * j + 32, n * 4 + q, 0:450],
                                        s_sb[32 * q:32 * q + 32, t, 32 * j:32 * j + 32],
                                        xv[32 * q:32 * q + 32, n, off:off + 450],
                                        start=(t == 0 and hl == 0),
                                        stop=(t == 8 and hl == 1),
                                        tile_position=(32 * q, 32 * j))
                # drain PSUM -> y_raw on ACT, bank pairs, skipping the 2-col seam
                for bp in range(4):  # bank pair (banks 2*bp, 2*bp+1)
                    n = bp // 2
                    q0 = (bp % 2) * 2
                    src = bass.AP(
                        tensor=pfull.tensor, offset=pfull.offset + (2 * bp) * 512,
                        ap=[[pstride, 128], [512, 2], [WP, 2], [1, 224]])
                    nc.scalar.copy(y_raw[:, n, q0:q0 + 2, s, :, :], src)
                for n in range(NPC):
                    for q in range(4):
                        nc.vector.bn_stats(
                            out=stats_buf[:, s, n * 4 + q, :],
                            in_=y_raw[:, n, q, s].rearrange("p i w -> p (i w)"))

            # ---- local stats -> (sum, sumsq) [128, 2] ----
            stats_fl = stats_buf.rearrange("p s b (e t) -> p (s b e) t", e=2, t=3)
            means = stats_fl[:, :, 1]
            ctv = stats_fl[:, :, 2]
            nc.vector.tensor_reduce(red[:, 0:1], means, mybir.AxisListType.X,
                                    mybir.AluOpType.add)
            nc.vector.tensor_tensor(msq_scr[:], means, means, mybir.AluOpType.mult)
            nc.vector.tensor_reduce(red[:, 1:2], msq_scr[:], mybir.AxisListType.X,
                                    mybir.AluOpType.add)
            nc.vector.tensor_reduce(red[:, 2:3], ctv, mybir.AxisListType.X,
                                    mybir.AluOpType.add)
            nc.vector.tensor_scalar_mul(stats_sq[:, 0:1], red[:, 0:1], 224.0)
            nc.vector.tensor_scalar_mul(red[:, 3:4], red[:, 1:2], 224.0)
            nc.vector.tensor_tensor(stats_sq[:, 1:2], red[:, 3:4], red[:, 2:3],
                                    mybir.AluOpType.add)

            # ---- all-reduce over 8 cores ----
            cc_in = dram.tile([128, 2], F32)
            cc_out = dram.tile([128, 2], F32)
            nc.gpsimd.dma_start(out=cc_in[:], in_=stats_sq[:])
            nc.gpsimd.collective_compute(
                "AllReduce", mybir.AluOpType.add,
                replica_groups=[list(range(N_CORES))],
                ins=[cc_in.opt()], outs=[cc_out.opt()])
            nc.gpsimd.dma_start(out=stats_g[:], in_=cc_out[:])

            # ---- combine j groups: [128,2] -> [32,2] via PE ----
            nc.tensor.matmul(psum_t[0:32, 0, 0:2], sel1_sb[:], stats_g[:],
                             start=True, stop=True)
            nc.scalar.copy(t32[:], psum_t[0:32, 0, 0:2])

            # ---- finalize per-channel scale/shift on partitions 0..31 ----
            mean = fin[:, 0:1]
            msqm = fin[:, 1:2]
            v = fin[:, 2:3]
            rec = fin[:, 3:4]
            a_ = fin[:, 4:5]
            bq = fin[:, 5:6]
            cq = fin[:, 6:7]
            sc = fin[:, 7:8]
            inv_n = float(np.float32(1.0) / np.float32(NTOT))
            nc.vector.tensor_scalar_mul(mean, t32[:, 0:1], inv_n)
            nc.vector.tensor_scalar_mul(msqm, t32[:, 1:2], inv_n)
            nc.vector.tensor_tensor(v, mean, mean, mybir.AluOpType.mult)
            nc.vector.tensor_tensor(v, msqm, v, mybir.AluOpType.subtract)
            nc.vector.tensor_scalar_add(v, v, EPS)
            nc.scalar.activation(rec, v, mybir.ActivationFunctionType.Sqrt)
            nc.vector.reciprocal(rec, rec)
            for _ in range(2):  # Newton polish: rec *= 1.5 - 0.5*v*rec^2
                nc.vector.tensor_tensor(a_, rec, rec, mybir.AluOpType.mult)
                nc.vector.tensor_tensor(bq, v, a_, mybir.AluOpType.mult)
                nc.vector.tensor_scalar(cq, bq, -0.5, 1.5, mybir.AluOpType.mult,
                                        mybir.AluOpType.add)
                nc.vector.tensor_tensor(rec, rec, cq, mybir.AluOpType.mult)
            nc.vector.tensor_tensor(sc, g_sb[:], rec, mybir.AluOpType.mult)
            nc.vector.tensor_scalar_mul(sb32[:, 0:1], sc, 2.0)
            nc.vector.tensor_tensor(a_, mean, sc, mybir.AluOpType.mult)
            nc.vector.tensor_tensor(bq, b_sb[:], a_, mybir.AluOpType.subtract)
            nc.vector.tensor_scalar_mul(sb32[:, 1:2], bq, 2.0)

            # broadcast [32,2] -> [128,2]
            nc.tensor.matmul(psum_t[:, 1, 0:2], sel2_sb[:], sb32[:],
                             start=True, stop=True)
            nc.scalar.copy(ab128[:], psum_t[:, 1, 0:2])

            # ---- phase 2: normalize + quantize + writeback ----
            yv = y_ext.ap().rearrange("n c (q s j i) w -> n c q s j i w",
                                      q=4, s=NSUP, j=4, i=2)
            for n in range(NPC):
                for q in range(4):
                    zin = y_raw[:, n, q].rearrange("p s i w -> p (s i w)")
                    u = ph2.tile([128, NSUP * 2 * W], F32, tag="u")
                    nc.scalar.activation(u[:], zin,
                                         mybir.ActivationFunctionType.Identity,
                                         bias=ab128[:, 1:2], scale=ab128[:, 0:1])
                    nc.vector.tensor_scalar(u[:], u[:], MAGIC, MAGIC + 2.0,
                                            mybir.AluOpType.add,
                                            mybir.AluOpType.min)
                    o = ph2.tile([128, NSUP * 2 * W], BF16, tag="o")
                    nc.vector.tensor_scalar(o[:], u[:], MAGIC - 2.0, MAGIC,
                                            mybir.AluOpType.max,
                                            mybir.AluOpType.subtract)
                    ov = o.rearrange("p (s i w) -> p s i w", s=NSUP, i=2)
                    for j in range(4):
                        nc.sync.dma_start(out=yv[n, :, q, :, j, :, :],
                                          in_=ov[32 * j:32 * j + 32])
    nc.compile()
    return nc


def _get_nc():
    if "nc" not in _cache:
        _cache["nc"] = _build_nc()
    return _cache["nc"]


def _host_consts(weight):
    w_bin = np.where(np.asarray(weight, dtype=np.float32) >= 0, 1.0, -1.0)
    # S[32q+ci, t, 32j+co] = w_bin[co, ci, kh, kw] with t = kh*3+kw
    s_np = np.zeros((128, 9, 128), dtype=ml_dtypes.bfloat16)
    wt = np.transpose(w_bin.reshape(C, C, 9), (1, 2, 0))  # [ci, t, co]
    wt = wt.astype(ml_dtypes.bfloat16)
    for qq in range(4):
        for jj in range(4):
            s_np[32 * qq:32 * qq + 32, :, 32 * jj:32 * jj + 32] = wt
    p = np.arange(128)
    sel1 = (p[:, None] % 32 == np.arange(32)[None, :]).astype(np.float32)
    sel2 = (np.arange(32)[:, None] == p[None, :] % 32).astype(np.float32)
    return s_np, sel1, sel2


def make_in_maps(x, weight, gamma, beta):
    x = np.asarray(x, dtype=np.float32)
    xh = x.astype(ml_dtypes.bfloat16)
    xl = (x - xh.astype(np.float32)).astype(ml_dtypes.bfloat16)
    s_np, sel1, sel2 = _host_consts(weight)
    g = np.asarray(gamma, dtype=np.float32).reshape(32, 1)
    b = np.asarray(beta, dtype=np.float32).reshape(32, 1)
    in_maps = []
    for c in range(N_CORES):
        sl = slice(c * NPC, (c + 1) * NPC)
        in_maps.append({"xh": xh[sl], "xl": xl[sl], "s": s_np,
                        "sel1": sel1, "sel2": sel2, "g": g, "b": b})
    return in_maps


def kernel(x, weight, gamma, beta):
    nc = _get_nc()
    in_maps = make_in_maps(x, weight, gamma, beta)
    res = run_bass_kernel_spmd(nc, in_maps, list(range(N_CORES)))
    out = np.concatenate([res.results[c]["y"] for c in range(N_CORES)], axis=0)
    return out.astype(np.float32) * 0.5
